# revision 26
# baseline (speedup 1.0000x reference)
"""
DeepAttMISL segment-reduce kernel for Trainium2 (Bass/Tile), 8 NeuronCores.

Math (see reference):
  h        = relu(x @ W1.T + b1)                    x:[N,1024] -> h:[N,256]
  seg      = segment_sum(h, cluster_id, 8)          -> [8,256]
  h_clust  = seg / max(counts,1)
  h_path   = relu(h_clust @ Wf.T + bf)
  A        = softmax((tanh(h_path@Wa.T+ba) * sigmoid(h_path@Wb.T+bb)) @ Wc.T + bc)
  H        = A @ h_path                             -> [1,256]

Sharding: rows are sorted by cluster on the host and core c gets exactly
cluster c's rows, zero-padded to a common N_PAD (multiple of 128).  The
big matmul runs weight-stationary (W1 128x128 chunks stationary, x rows
streaming 512 wide) so LDWEIGHTS is trivially hidden and the output lands
transposed ([hid, row]) in PSUM.  The ACT engine applies bias+relu (bias
is per-partition in this layout), the DVE reduces each 128-row sub-block,
and one more DVE reduce gives this core's cluster sum [256] directly in
the transposed layout the attention head wants.  Each core scatters its
sum into its one-hot column of a [128,2,8] buffer; an AllReduce over the
8 cores assembles the full per-cluster sums.  Zero-pad rows contribute
relu(b1) each; that is corrected after the AllReduce by subtracting
nz_k * relu(b1) (relu(b1) computed on device).

The head replaces sigmoid(z) with 0.5*tanh(z/2)+0.5 so relu/tanh/exp all
live in one ACT table set; dummy exp+tanh activations at kernel start
prepay the table load.

Precision: the big matmul runs in bf16 (inputs rounded once on host) with
fp32 PSUM accumulation; everything from the segment sums onward is fp32.
The softmax skips the max-subtraction: its logits are bounded.
"""

import sys

if "/opt/trn_rl_repo" not in sys.path:
    sys.path.insert(0, "/opt/trn_rl_repo")

from contextlib import ExitStack

import numpy as np
import ml_dtypes

import concourse.bass as bass
import concourse.tile as tile
from concourse import bacc, mybir
from concourse import bass_utils

N_CORES = 8
N_TOTAL = 65536
DIN = 1024
DHID = 256
K_CL = 8                               # clusters (== cores)
KC = DIN // 128                        # 8 contraction chunks

BF16 = mybir.dt.bfloat16
F32 = mybir.dt.float32
AF = mybir.ActivationFunctionType
ALU = mybir.AluOpType

_CACHE = {}


def _slab_sizes(n_pad):
    """Row-slab sizes for the x DMAs: small first so the PE starts early."""
    sizes = []
    rem = n_pad
    for s in (512, 512):
        if rem <= s:
            break
        sizes.append(s)
        rem -= s
    while rem > 1536:
        sizes.append(1024)
        rem -= 1024
    if rem > 0:
        sizes.append(rem)
    return sizes


# packed small-const layout: columns in the cst [128, CST_W] f32 tensor
CST_B1T = 0        # [128, 2]   b1 transposed
CST_OH = 2         # [128, 8]   one-hot core column
CST_NZT = 10       # [128, 2, 8] pad counts per cluster (replicated)
CST_INVT = 26      # [128, 2, 8] 1/max(counts,1)   (replicated)
CST_BFC = 42       # [128, 2]
CST_BAC = 44       # [128, 2]
CST_BBCH = 46      # [128, 2]   bb/2
CST_BCR = 48       # [128, 1]
CST_W = 49


def _build_nc(n_pad):
    slabs = _slab_sizes(n_pad)
    nsub = n_pad // 128

    nc = bacc.Bacc("TRN2", target_bir_lowering=False, debug=False,
                   num_devices=N_CORES)

    # ---- per-core external inputs (all pre-rearranged on the host so every
    # DMA has one contiguous >=2KB run per partition) ----
    xs_dram = [nc.dram_tensor(f"xs{i}", [128, KC, R], BF16, kind="ExternalInput")
               for i, R in enumerate(slabs)]
    w1p = nc.dram_tensor("w1p", [128, KC, DHID], BF16, kind="ExternalInput")
    cst = nc.dram_tensor("cst", [128, CST_W], F32, kind="ExternalInput")
    whd = nc.dram_tensor("whd", [128, 2, 3 * DHID + 128], mybir.dt.float32r,
                         kind="ExternalInput")

    out = nc.dram_tensor("out", [1, DHID], F32, kind="ExternalOutput")

    with tile.TileContext(nc) as tc, ExitStack() as stack:
        consts = stack.enter_context(tc.tile_pool(name="consts", bufs=1))
        xpools = [stack.enter_context(tc.tile_pool(name=f"x{i}", bufs=1))
                  for i in range(len(slabs))]
        hpool = stack.enter_context(tc.tile_pool(name="hpool", bufs=4))
        hps = stack.enter_context(tc.tile_pool(name="hps", bufs=4, space="PSUM"))
        headps = stack.enter_context(tc.tile_pool(name="headps", bufs=2, space="PSUM"))
        small = stack.enter_context(tc.tile_pool(name="small", bufs=1))
        dram = stack.enter_context(tc.tile_pool(name="dram", bufs=1, space="DRAM"))

        # ---- critical-path loads: w1 on the ACT ring, packed consts + first
        # x slab on the SP ring; remaining slabs alternate.  w1 and the first
        # slab are loaded per-k-chunk so the k=0 matmul can fire as soon as
        # its own chunk lands instead of waiting for the full tile. ----
        w1s = consts.tile([128, KC, DHID], BF16)
        nc.scalar.dma_start(w1s[:, 0:1, :], w1p.ap()[:, 0:1, :])
        nc.scalar.dma_start(w1s[:, 1:2, :], w1p.ap()[:, 1:2, :])
        nc.scalar.dma_start(w1s[:, 2:KC, :], w1p.ap()[:, 2:KC, :])
        cst_sb = consts.tile([128, CST_W], F32)
        nc.scalar.dma_start(cst_sb[:], cst.ap())
        b1T_sb = cst_sb[:, CST_B1T:CST_B1T + 2]
        oh_sb = cst_sb[:, CST_OH:CST_OH + K_CL]

        xts = []
        for i, R in enumerate(slabs):
            t = xpools[i].tile([128, KC, R], BF16, name=f"xts{i}")
            xts.append(t)
            if i == 0:
                for k in range(KC):
                    nc.sync.dma_start(t[:, k, :], xs_dram[0].ap()[:, k, :])
            else:
                eng = nc.sync if i % 2 == 0 else nc.scalar
                eng.dma_start(t[:], xs_dram[i].ap())

        # ---- ACT table prepay: dummy exp+tanh before the relu stream ----
        dum = small.tile([1, 2], F32)
        nc.vector.memset(dum[:], 0.0)
        dum2 = small.tile([1, 2], F32)
        nc.scalar.activation(dum2[0:1, 0:1], dum[0:1, 0:1], AF.Exp)
        nc.scalar.activation(dum2[0:1, 1:2], dum[0:1, 0:1], AF.Tanh)



        # ---- head weights (packed, off the critical path) ----
        whd_sb = consts.tile([128, 2, 3 * DHID + 128], mybir.dt.float32r)
        nc.sync.dma_start(whd_sb[:], whd.ap())
        wft_sb = whd_sb[:, :, 0:DHID]
        wat_sb = whd_sb[:, :, DHID:2 * DHID]
        wbt_sb = whd_sb[:, :, 2 * DHID:3 * DHID]
        wcr_sb = whd_sb[:, :, 3 * DHID:3 * DHID + 128]
        bfc_sb = cst_sb[:, CST_BFC:CST_BFC + 2]
        bac_sb = cst_sb[:, CST_BAC:CST_BAC + 2]
        bbch_sb = cst_sb[:, CST_BBCH:CST_BBCH + 2]
        bcr_sb = cst_sb[:, CST_BCR:CST_BCR + 1]

        # ---- main loop: weight-stationary matmul + fused bias/relu +
        # per-sub-block sums ----
        p_sb = consts.tile([128, 2, nsub], F32)
        bidx = 0
        for s, R in enumerate(slabs):
            local = 0
            while local < R:
                rb = min(512, R - local)
                nb = rb // 128
                for j in range(2):
                    ps = hps.tile([128, 4, 128], F32, tag="ps")
                    for k in range(KC):
                        nc.tensor.matmul(
                            ps[:, :nb, :],
                            w1s[:, k, j * 128:(j + 1) * 128],
                            xts[s][:, k, local:local + rb],
                            start=(k == 0), stop=(k == KC - 1),
                            skip_group_check=True)
                    h = hpool.tile([128, 4, 128], BF16, tag="h")
                    nc.scalar.activation(h[:, :nb, :], ps[:, :nb, :], AF.Relu,
                                         bias=b1T_sb[:, j:j + 1])
                    nc.vector.reduce_sum(p_sb[:, j, bidx:bidx + nb], h[:, :nb, :],
                                         axis=mybir.AxisListType.X)
                local += rb
                bidx += nb
        assert bidx == nsub

        # ---- this core's cluster sum, scattered to its one-hot column ----
        s2 = small.tile([128, 2], F32)
        hcT_part = small.tile([128, 2, K_CL], F32)
        for j in range(2):
            nc.vector.reduce_sum(s2[:, j:j + 1], p_sb[:, j, :],
                                 axis=mybir.AxisListType.X)
            nc.vector.tensor_scalar_mul(hcT_part[:, j, :], oh_sb,
                                        s2[:, j:j + 1])

        # ---- AllReduce partial segment sums across the 8 cores ----
        ar_in = dram.tile([128, 2, K_CL], F32)
        ar_out = dram.tile([128, 2, K_CL], F32)
        nc.sync.dma_start(ar_in[:], hcT_part[:])
        nc.gpsimd.collective_compute(
            "AllReduce", ALU.add,
            replica_groups=[list(range(N_CORES))],
            ins=[ar_in[:].opt()], outs=[ar_out[:].opt()])
        ar_sb = small.tile([128, 2, K_CL], F32)
        nc.sync.dma_start(ar_sb[:], ar_out[:])

        # ---- zero-pad correction + cluster means (transposed layout) ----
        # The head matmuls run in float32r with the moving operand padded to
        # 256 columns (zeros beyond the 8 clusters): fp32r streams at 1
        # cycle/row when the moving dim is >=256, 4x faster than fp32.
        NP = 256
        F32R = mybir.dt.float32r
        rb1 = small.tile([128, 2], F32)
        nc.scalar.activation(rb1[:], b1T_sb, AF.Relu)
        corr = small.tile([128, 2, K_CL], F32)
        hcT = small.tile([128, 2, NP], F32R)
        hpT = small.tile([128, 2, NP], F32R)
        agT = small.tile([128, 2, NP], F32R)
        for t in (hcT, hpT, agT):
            nc.vector.memset(t[:].bitcast(F32), 0.0)
        for j in range(2):
            nc.vector.tensor_scalar_mul(
                corr[:, j, :],
                cst_sb[:, CST_NZT + j * K_CL:CST_NZT + (j + 1) * K_CL],
                rb1[:, j:j + 1])
            nc.vector.tensor_sub(hcT[:, j, 0:K_CL], ar_sb[:, j, :],
                                 corr[:, j, :])
            nc.vector.tensor_mul(
                hcT[:, j, 0:K_CL], hcT[:, j, 0:K_CL],
                cst_sb[:, CST_INVT + j * K_CL:CST_INVT + (j + 1) * K_CL])


        def head_mm(wt_sb, rhs, bias_sb, func, out, scale=1.0):
            for j in range(2):
                ps = headps.tile([128, NP], F32, tag="head")
                for i in range(2):
                    nc.tensor.matmul(
                        ps[:],
                        wt_sb[:, i, j * 128:(j + 1) * 128],
                        rhs[:, i, :],
                        start=(i == 0), stop=(i == 1))
                nc.scalar.activation(out[:, j, 0:K_CL], ps[:, 0:K_CL], func,
                                     bias=bias_sb[:, j:j + 1], scale=scale)
            return out

        head_mm(wft_sb, hcT, bfc_sb, AF.Relu, hpT)
        aT = small.tile([128, 2, K_CL], F32)
        head_mm(wat_sb, hpT, bac_sb, AF.Tanh, aT)
        # sigmoid(z) = 0.5*tanh(z/2) + 0.5 ; bbch is pre-halved on the host
        tT = small.tile([128, 2, K_CL], F32)
        head_mm(wbt_sb, hpT, bbch_sb, AF.Tanh, tT, scale=0.5)
        t1 = small.tile([128, 2, K_CL], F32)
        nc.vector.tensor_scalar_add(t1, tT, 1.0)
        nc.vector.tensor_mul(agT[:, :, 0:K_CL], aT, t1)  # = 2*a*g; wcr halved

        a_ps = headps.tile([128, NP], F32, tag="head")
        for j in range(2):
            nc.tensor.matmul(a_ps[:], wcr_sb[:, j, :],
                             agT[:, j, :],
                             start=(j == 0), stop=(j == 1))
        a_sb = small.tile([128, K_CL], F32)
        nc.vector.tensor_scalar_add(a_sb[:], a_ps[:, 0:K_CL], bcr_sb[:, 0:1])

        # softmax over the 8 clusters (bounded logits; skip max-shift)
        ea = small.tile([128, K_CL], F32)
        nc.scalar.activation(ea[:], a_sb[:], AF.Exp)
        ssum = small.tile([128, 1], F32)
        nc.vector.reduce_sum(ssum[:], ea[:], axis=mybir.AxisListType.X)
        rs = small.tile([128, 1], F32)
        nc.vector.reciprocal(rs[:], ssum[:])
        an = small.tile([128, K_CL], F32)
        nc.vector.tensor_scalar_mul(an[:], ea[:], rs[:, 0:1])

        # H[hid] = sum_k A[k] * h_path.T[hid, k]
        h_out = small.tile([128, 2], F32)
        for j in range(2):
            tmp = small.tile([128, K_CL], F32, name=f"wtmp{j}")
            nc.vector.tensor_mul(tmp[:], hpT[:, j, 0:K_CL], an[:])
            nc.vector.reduce_sum(h_out[:, j:j + 1], tmp[:],
                                 axis=mybir.AxisListType.X)
        nc.sync.dma_start(out.ap().rearrange("a (j p) -> p a j", p=128),
                          h_out[:])

    nc.compile()
    return nc


def _layout(cluster_id):
    """Global sort by cluster; core c owns cluster c, padded to n_pad rows."""
    cid = np.asarray(cluster_id).astype(np.int64).reshape(N_TOTAL)
    counts = np.bincount(cid, minlength=K_CL).astype(np.int64)
    n_pad = int(((counts.max() + 127) // 128) * 128)
    n_pad = max(n_pad, 512)
    order = np.argsort(cid, kind="stable")
    bounds = np.concatenate([[0], np.cumsum(counts)])
    rows = [order[bounds[c]:bounds[c + 1]] for c in range(K_CL)]
    return counts, n_pad, rows


def _prep_inputs(x_path, cluster_id, W1, b1, Wf, bf, Wa, ba, Wb, bb, Wc, bc):
    """Host-side sharding / marshalling. Returns in_maps for the 8 cores."""
    x = np.asarray(x_path, dtype=np.float32).reshape(N_TOTAL, DIN)
    counts, n_pad, rows = _layout(cluster_id)
    slabs = _slab_sizes(n_pad)

    xb = x.astype(ml_dtypes.bfloat16)

    countsf = counts.astype(np.float32)
    invc = (1.0 / np.maximum(countsf, 1.0)).astype(np.float32)
    nz = (n_pad - countsf).astype(np.float32)

    W1 = np.asarray(W1, np.float32); b1 = np.asarray(b1, np.float32)
    Wf = np.asarray(Wf, np.float32); bf = np.asarray(bf, np.float32)
    Wa = np.asarray(Wa, np.float32); ba = np.asarray(ba, np.float32)
    Wb = np.asarray(Wb, np.float32); bb = np.asarray(bb, np.float32)
    Wc = np.asarray(Wc, np.float32); bc = np.asarray(bc, np.float32)

    # w1 pre-rearranged: w1p[p, k, h] = W1.T[k*128+p, h]
    w1p = np.ascontiguousarray(
        W1.T.reshape(KC, 128, DHID).transpose(1, 0, 2)).astype(ml_dtypes.bfloat16)

    # head weights packed: whd[p, i, :] = [WfT, WaT, WbT, 0.5*WcT] rows i*128+p
    def _rr(w):
        return w.T.reshape(2, 128, -1).transpose(1, 0, 2)
    wcr = np.broadcast_to(Wc.T, (DHID, 128)) * 0.5
    whd = np.ascontiguousarray(np.concatenate(
        [_rr(Wf), _rr(Wa), _rr(Wb), _rr(wcr.T)], axis=2).astype(np.float32))

    cst0 = np.zeros((128, CST_W), np.float32)
    cst0[:, CST_B1T:CST_B1T + 2] = b1.reshape(2, 128).T
    cst0[:, CST_NZT:CST_NZT + 2 * K_CL] = np.tile(nz, 2)[None, :]
    cst0[:, CST_INVT:CST_INVT + 2 * K_CL] = np.tile(invc, 2)[None, :]
    cst0[:, CST_BFC:CST_BFC + 2] = bf.reshape(2, 128).T
    cst0[:, CST_BAC:CST_BAC + 2] = ba.reshape(2, 128).T
    cst0[:, CST_BBCH:CST_BBCH + 2] = bb.reshape(2, 128).T * 0.5
    cst0[:, CST_BCR] = float(bc.ravel()[0])

    in_maps = []
    for c in range(N_CORES):
        xpc = np.zeros((n_pad, DIN), dtype=ml_dtypes.bfloat16)
        xpc[:counts[c]] = xb[rows[c]]
        cmap = {"w1p": w1p, "whd": whd}
        r0 = 0
        for i, R in enumerate(slabs):
            blk = np.ascontiguousarray(
                xpc[r0:r0 + R].T.reshape(KC, 128, R).transpose(1, 0, 2))
            cmap[f"xs{i}"] = blk
            r0 += R
        cstc = cst0.copy()
        cstc[:, CST_OH + c] = 1.0
        cmap["cst"] = cstc
        in_maps.append(cmap)
    return in_maps


def kernel(**inputs):
    _, n_pad, _ = _layout(inputs["cluster_id"])
    key = ("nc", n_pad)
    if key not in _CACHE:
        _CACHE[key] = _build_nc(n_pad)
        _CACHE["nc"] = _CACHE[key]
    nc = _CACHE[key]
    _CACHE["nc"] = nc
    in_maps = _prep_inputs(**inputs)
    res = bass_utils.run_bass_kernel_spmd(
        nc, in_maps, core_ids=list(range(N_CORES)))
    return res.results[0]["out"].astype(np.float32)
